# revision 40
# baseline (speedup 1.0000x reference)
"""Trainium2 Bass kernel for the neural-ODE Euler integration problem.

v2 design:
- Pure data parallel: 128 trajectories/core, split into G=2 groups of 64
  that pipeline against each other (engines overlap across groups).
- a1-recurrence: the layer-1 pre-activation a1 = x@Ux + zeff@Uz + allinit@V
  + b1 lives in a persistent PSUM bank per group.  Each step the PE
  accumulates  dt*dx@Ux + zeff_{i+1}@Uz - zeff_i@Uz  via ONE K=24 matmul
  per hidden half whose rhs block [xdelta; zeff_{i+1}; zeff_i] is staged
  in SBUF (zeff rows DMA'd from DRAM, xdelta written by the scalar engine).
  x itself is never materialized on-chip.
- Output: the chip emits xdelta_i = S*dt_i*dx_i (fp16); the host computes
  x = x0 + cumsum(xdelta)/S in numpy.
- elu(a)+1 = relu(a) + min(exp(a),1): r on vector, e on scalar, m on
  vector; r/m feed matmul pairs sharing stationary weights (the +1 is
  corrected in biases: b_eff = b - colsum(W)).
- z_eff (event switch) precomputed in numpy on the host.
- Hidden halves packed in PSUM tile columns [128, 2*64]; biases enter via
  tiny K=4 (hi/lo x halves) matmuls against a static selector rhs.
"""

import math
import numpy as np
import sys

if '/opt/trn_rl_repo' not in sys.path:
    sys.path.insert(0, '/opt/trn_rl_repo')

import concourse.bass as bass
import concourse.bacc as bacc
import concourse.mybir as mybir
from concourse.tile import TileContext
from concourse import bass_utils

F32 = mybir.dt.float32
F16 = mybir.dt.float16
AF = mybir.ActivationFunctionType

B, T, XD, ZD, HID = 1024, 1000, 8, 8, 256
NCORES = 8
PB = B // NCORES          # 128 trajectories per core
G = 2                     # pipelined groups per core
GB = PB // G              # 64 trajectories per group
CH = 64                   # steps per z/xdelta chunk
NSTEPS = T - 1
SCALE = 64.0              # xdelta fp16 scaling

LAST_RESULTS = None


def _build(nsteps):
    nc = bacc.Bacc("TRN2", target_bir_lowering=False, debug=False,
                   num_devices=NCORES)
    nchunks = (nsteps + CH - 1) // CH
    nslot = nchunks * CH

    d = {}
    def din(name, shape, dt):
        d[name] = nc.dram_tensor(name, shape, dt, kind="ExternalInput").ap()

    for g in range(G):
        din(f"zpair{g}", [17, nslot * GB], F16)
        din(f"initr{g}", [26, GB], F16)
        din(f"z0r{g}", [8, GB], F16)
    din("updh0", [128, 128], F16)
    din("updh1", [128, 128], F16)
    din("inith0", [26, 128], F16)
    din("inith1", [26, 128], F16)
    din("uzh0", [8, 128], F16)
    din("uzh1", [8, 128], F16)
    din("w2p", [128, 512], F16)
    din("w3p", [128, 512], F16)
    din("w41p", [128, 512], F16)   # (mean_dt * W4 @ Ux) packed like w2p
    din("b2cat", [128, 128], F16)
    din("b3cat", [128, 128], F16)
    din("sel4", [128, 128], F16)
    hout_d = nc.dram_tensor("hout_d", [128, nslot, G, 128], F16,
                            kind="ExternalOutput").ap()

    with TileContext(nc) as tc:
        with tc.tile_pool(name="const", bufs=1) as cpool, \
             tc.tile_pool(name="work", bufs=2) as wpool, \
             tc.tile_pool(name="psum", bufs=1, space="PSUM") as ppool:

            sb = {}
            for name in d:
                if name.startswith("zpair"):
                    continue        # stays in DRAM; chunk-DMA'd below
                shape = [int(s) for s in d[name].shape]
                sb[name] = cpool.tile(shape, d[name].dtype, name=name,
                                      tag=name)
                nc.sync.dma_start(out=sb[name][:], in_=d[name])

            # persistent PSUM banks + stream tiles per group; a2 rotates
            # over 2 banks (8 total: 2 a1 + 4 a2 + 2 a3) so next step's
            # layer-2 bias matmul never waits on this step's elu2 reads
            a1, a2, a3 = {}, {}, {}
            streams = {}
            for g in range(G):
                a1[g] = ppool.tile([128, 128], F32, name=f"a1g{g}",
                                   tag=f"a1g{g}")
                a3[g] = ppool.tile([128, 128], F32, name=f"a3g{g}",
                                   tag=f"a3g{g}")
                for l in (1, 2, 3):
                    for s in ("r", "e", "m"):
                        if l == 3 and s == "m":
                            continue
                        streams[(g, l, s)] = wpool.tile(
                            [128, 128], F16, name=f"{s}{l}g{g}",
                            tag=f"{s}{l}g{g}", bufs=1)

            def emit_step(g, i, blk_tile, blk, cur_tile, i_loc, part):
                a1g = a1[g]
                if part == 1:
                    a2[g] = ppool.tile([128, 128], F32, name=f"a2g{g}",
                                       tag=f"a2g{g}", bufs=2)
                if part == 1 and i == 0:
                    nc.tensor.matmul(a1g[:, 0:GB], lhsT=sb["inith0"][:],
                                     rhs=sb[f"initr{g}"][:],
                                     start=True, stop=False,
                                     skip_group_check=True)
                    nc.tensor.matmul(a1g[:, GB:2 * GB], lhsT=sb["inith1"][:],
                                     rhs=sb[f"initr{g}"][:],
                                     start=False, stop=False,
                                     skip_group_check=True)
                    nc.tensor.matmul(a1g[:, 0:GB], lhsT=sb["uzh0"][:],
                                     rhs=sb[f"z0r{g}"][:],
                                     start=False, stop=False,
                                     skip_group_check=True)
                    nc.tensor.matmul(a1g[:, GB:2 * GB], lhsT=sb["uzh1"][:],
                                     rhs=sb[f"z0r{g}"][:],
                                     start=False, stop=True,
                                     skip_group_check=True)
                elif part == 1 and i > 0:
                    rb = blk_tile[0:128, blk * GB:(blk + 1) * GB]
                    nc.tensor.matmul(a1g[:, 0:GB], lhsT=sb["updh0"][:],
                                     rhs=rb, start=False, stop=False,
                                     skip_group_check=True)
                    nc.tensor.matmul(a1g[:, GB:2 * GB], lhsT=sb["updh1"][:],
                                     rhs=rb, start=False, stop=True,
                                     skip_group_check=True)

                # layer 1..3: streams + next-layer matmuls
                # h' = elu(a)+1 = relu(a) + min(exp(a),1); computed as
                # r (vector), e (scalar), then h' = (e min 1) + r (vector stt)
                layers = ((a1g, a2[g], "w2p", "b2cat"),
                          (a2[g], a3[g], "w3p", "b3cat"),
                          (a3[g], None, "w4p", None))
                sel = (1,) if part == 1 else (2, 3)
                for l, (ain, aout, wname, bname) in enumerate(layers, start=1):
                    if l not in sel:
                        continue
                    r = streams[(g, l, "r")]
                    e = streams[(g, l, "e")]
                    if l == 3:
                        hh = wpool.tile([128, 128], F16, name=f"m3g{g}",
                                        tag=f"m3g{g}", bufs=2)
                    else:
                        hh = streams[(g, l, "m")]
                    if l == 1:
                        # vector is the busiest engine; scalar (relu shares
                        # exp's table set) takes layer 1's evacuation
                        nc.scalar.activation(r[:], ain[:], AF.Relu)
                    else:
                        nc.vector.tensor_scalar_max(r[:], ain[:], 0.0)
                    nc.scalar.activation(e[:], ain[:], AF.Exp)
                    nc.vector.scalar_tensor_tensor(
                        out=hh[:], in0=e[:], scalar=1.0, in1=r[:],
                        op0=mybir.AluOpType.min, op1=mybir.AluOpType.add)
                    if aout is not None:
                        nc.tensor.matmul(aout[:, 0:128], lhsT=sb[bname][:],
                                         rhs=sb["sel4"][:],
                                         start=True, stop=False)
                        for h in range(2):
                            hs = slice(h * GB, (h + 1) * GB)
                            for kc in range(2):
                                lh = sb[wname][:, (kc * 2 + h) * 128:
                                               (kc * 2 + h + 1) * 128]
                                ks = slice(kc * GB, (kc + 1) * GB)
                                nc.tensor.matmul(aout[:, hs], lhsT=lh,
                                                 rhs=hh[:, ks],
                                                 start=False,
                                                 stop=(h == 1 and kc == 1))
                    else:
                        # critical path: feed x-increment straight into the
                        # a1 recurrence via W41 = mean_dt*W4@Ux; layer 4
                        # itself runs on the host from the shipped h3'
                        for h in range(2):
                            hs = slice(h * GB, (h + 1) * GB)
                            for kc in range(2):
                                lh = sb["w41p"][:, (kc * 2 + h) * 128:
                                                (kc * 2 + h + 1) * 128]
                                ks = slice(kc * GB, (kc + 1) * GB)
                                nc.tensor.matmul(
                                    a1g[:, hs], lhsT=lh, rhs=hh[:, ks],
                                    start=False, stop=False,
                                    skip_group_check=True)
                        nc.sync.dma_start(out=hout_d[:, i, g, :],
                                          in_=hh[:])

            zxt = {g: None for g in range(G)}
            prev = {g: None for g in range(G)}
            for c in range(nchunks):
                for g in range(G):
                    prev[g] = zxt[g]
                    zxt[g] = wpool.tile([128, CH * GB], F16, name=f"zx{g}",
                                        tag=f"zx{g}", bufs=2)
                    if c < 2:
                        # rows 25-127 must be zero for the K=128 update
                        # matmul; clear the whole slot once (real rows are
                        # overwritten by the DMA below / scalar per step)
                        nc.vector.memset(zxt[g][:], 0)
                    nc.sync.dma_start(
                        out=zxt[g][8:25, :],
                        in_=d[f"zpair{g}"][:, c * CH * GB:(c + 1) * CH * GB])
                for i_loc in range(CH):
                    i = c * CH + i_loc
                    if i >= nsteps:
                        break
                    if i_loc == 0:
                        bt0, blk0 = prev[0], CH - 1
                        bt1, blk1 = prev[1], CH - 1
                    else:
                        bt0, blk0 = zxt[0], i_loc - 1
                        bt1, blk1 = zxt[1], i_loc - 1
                    # stagger group 1 half a step behind group 0 so their
                    # serial chains anti-phase instead of colliding on the
                    # same engine at every layer
                    emit_step(0, i, bt0, blk0, zxt[0], i_loc, 1)
                    if i > 0:
                        pl = (i - 1) % CH
                        emit_step(1, i - 1, None, None, pzx1, pl, 2)
                    emit_step(0, i, bt0, blk0, zxt[0], i_loc, 2)
                    emit_step(1, i, bt1, blk1, zxt[1], i_loc, 1)
                    pzx1 = zxt[1]
            # tail: close group 1's final step
            last = nsteps - 1
            emit_step(1, last, None, None, pzx1, last % CH, 2)

    nc.compile()
    return nc


_BUILD_CACHE = {}


def _get_compiled(nsteps):
    if nsteps not in _BUILD_CACHE:
        _BUILD_CACHE[nsteps] = _build(nsteps)
    return _BUILD_CACHE[nsteps]


def _hilo(v):
    hi = v.astype(np.float16)
    lo = (v - hi.astype(np.float32)).astype(np.float16)
    return hi, lo


def kernel(t, x, z, event_t, z_jump, W1, b1, W2, b2, W3, b3, W4, b4,
           nsteps=NSTEPS):
    global LAST_RESULTS
    t = np.asarray(t, np.float32); x = np.asarray(x, np.float32)
    z = np.asarray(z, np.float32)
    event_t = np.asarray(event_t, np.float32)
    z_jump = np.asarray(z_jump, np.float32)
    W1 = np.asarray(W1, np.float32); b1 = np.asarray(b1, np.float32)
    W2 = np.asarray(W2, np.float32); b2 = np.asarray(b2, np.float32)
    W3 = np.asarray(W3, np.float32); b3 = np.asarray(b3, np.float32)
    W4 = np.asarray(W4, np.float32); b4 = np.asarray(b4, np.float32)

    nchunks = (nsteps + CH - 1) // CH
    nslot = nchunks * CH
    tv = t[0, :, 0]
    dt = (tv[1:nsteps + 1] - tv[:nsteps]).astype(np.float32)   # [nsteps]

    # weight-derived shared tensors
    W1a, W1b, W1c = W1[0:16], W1[16:32], W1[32:48]
    V = (W1a - W1b).astype(np.float32)          # [16, 256]
    U = (W1b + W1c).astype(np.float32)          # [16, 256]
    Ux, Uz = U[0:8], U[8:16]
    b1hi, b1lo = _hilo(b1)

    sel4 = np.zeros((128, 128), np.float16)
    sel4[0:2, 0:64] = 1.0
    sel4[2:4, 64:128] = 1.0
    shared = dict(w2p=W2.reshape(2, 128, 2, 128).transpose(1, 0, 2, 3)
                        .reshape(128, 512).astype(np.float16),
                  w3p=W3.reshape(2, 128, 2, 128).transpose(1, 0, 2, 3)
                        .reshape(128, 512).astype(np.float16),
                  sel4=sel4)
    b4eff = (b4 - W4.sum(0)).astype(np.float32)          # [8]
    b4ux = b4eff @ Ux                                    # [256]
    dtbar = np.float32(dt.astype(np.float64).mean())
    W41 = (dtbar * (W4 @ Ux)).astype(np.float32)         # [256, 256]
    shared["w41p"] = (W41.reshape(2, 128, 2, 128).transpose(1, 0, 2, 3)
                      .reshape(128, 512).astype(np.float16))
    for h in range(2):
        hs = slice(h * 128, (h + 1) * 128)
        upd = np.zeros((128, 128), np.float32)
        # rows 0-7 stay zero: the x-increment enters a1 via the fused
        # W41 matmuls; the zx xdelta rows are output-only
        upd[8:16] = Uz[:, hs]
        upd[16:24] = -Uz[:, hs]
        upd[24] = b4ux[hs] / SCALE
        shared[f"updh{h}"] = upd.astype(np.float16)
        init = np.concatenate([Ux[:, hs], V[:, hs],
                               b1hi[None, hs], b1lo[None, hs]], axis=0)
        shared[f"inith{h}"] = init.astype(np.float16)
        shared[f"uzh{h}"] = Uz[:, hs].astype(np.float16)
    for (Wm, bm, name) in ((W2, b2, "b2cat"), (W3, b3, "b3cat")):
        beff = bm - Wm.sum(0)
        hi, lo = _hilo(beff)
        cat = np.zeros((128, 128), np.float16)
        cat[0] = hi[0:128]; cat[1] = lo[0:128]
        cat[2] = hi[128:256]; cat[3] = lo[128:256]
        shared[name] = cat

    in_maps = []
    for c in range(NCORES):
        m = dict(shared)
        for g in range(G):
            bs = slice(c * PB + g * GB, c * PB + (g + 1) * GB)
            mask = tv[None, :nsteps] >= event_t[bs]            # [GB, nsteps]
            zeff = np.where(mask[..., None], z_jump[bs][:, None, :],
                            z[bs, :nsteps]).astype(np.float32)  # [GB,ns,8]
            zp = np.zeros((17, nslot, GB), np.float32)
            zp[8:16, :nsteps] = zeff.transpose(2, 1, 0)         # zeff_i
            zp[0:8, :nsteps - 1] = zeff[:, 1:].transpose(2, 1, 0)
            zp[16, :nsteps] = (SCALE * dt)[:, None]             # b4 row
            m[f"zpair{g}"] = zp.reshape(17, nslot * GB).astype(np.float16)
            x0 = x[bs, 0]                                       # [GB, 8]
            z0 = z[bs, 0]
            initr = np.concatenate(
                [x0.T, x0.T, z0.T, np.ones((2, GB), np.float32)], axis=0)
            m[f"initr{g}"] = initr.astype(np.float16)
            m[f"z0r{g}"] = zeff[:, 0].T.astype(np.float16)
        in_maps.append({k: np.ascontiguousarray(v) for k, v in m.items()})

    nc = _get_compiled(nsteps)
    res = bass_utils.run_bass_kernel_spmd(nc, in_maps,
                                          core_ids=list(range(NCORES)))
    LAST_RESULTS = res

    out = np.zeros((B, T, XD), np.float32)
    n = min(nsteps + 1, T)
    for c in range(NCORES):
        raw = res.results[c]["hout_d"]          # [128, nslot, G, 128] f16
        for g in range(G):
            bs = slice(c * PB + g * GB, c * PB + (g + 1) * GB)
            # cols = (half, batch): H[half*128+p, i, b] = raw[p, i, g, h*64+b]
            hg = raw[:, :nsteps, g, :].astype(np.float32)   # [128, ns, 128]
            H = np.concatenate([hg[:, :, 0:GB], hg[:, :, GB:2 * GB]],
                               axis=0)                      # [256, ns, GB]
            dx = np.einsum('kd,kib->dib', W4, H) + b4eff[:, None, None]
            xd = dx * dt[None, :, None]                     # [8, ns, GB]
            cum = np.cumsum(xd, axis=1)
            out[bs, 0] = x[bs, 0]
            out[bs, 1:n] = (x[bs, 0][:, None, :]
                            + cum.transpose(2, 1, 0)[:, :n - 1])
    return out


# revision 41
# speedup vs baseline: 1.0589x; 1.0589x over previous
"""Trainium2 Bass kernel for the neural-ODE Euler integration problem.

v2 design:
- Pure data parallel: 128 trajectories/core, split into G=2 groups of 64
  that pipeline against each other (engines overlap across groups).
- a1-recurrence: the layer-1 pre-activation a1 = x@Ux + zeff@Uz + allinit@V
  + b1 lives in a persistent PSUM bank per group.  Each step the PE
  accumulates  dt*dx@Ux + zeff_{i+1}@Uz - zeff_i@Uz  via ONE K=24 matmul
  per hidden half whose rhs block [xdelta; zeff_{i+1}; zeff_i] is staged
  in SBUF (zeff rows DMA'd from DRAM, xdelta written by the scalar engine).
  x itself is never materialized on-chip.
- Output: the chip emits xdelta_i = S*dt_i*dx_i (fp16); the host computes
  x = x0 + cumsum(xdelta)/S in numpy.
- elu(a)+1 = relu(a) + min(exp(a),1): r on vector, e on scalar, m on
  vector; r/m feed matmul pairs sharing stationary weights (the +1 is
  corrected in biases: b_eff = b - colsum(W)).
- z_eff (event switch) precomputed in numpy on the host.
- Hidden halves packed in PSUM tile columns [128, 2*64]; biases enter via
  tiny K=4 (hi/lo x halves) matmuls against a static selector rhs.
"""

import math
import numpy as np
import sys

if '/opt/trn_rl_repo' not in sys.path:
    sys.path.insert(0, '/opt/trn_rl_repo')

import concourse.bass as bass
import concourse.bacc as bacc
import concourse.mybir as mybir
from concourse.tile import TileContext
from concourse import bass_utils

F32 = mybir.dt.float32
F16 = mybir.dt.float16
AF = mybir.ActivationFunctionType

B, T, XD, ZD, HID = 1024, 1000, 8, 8, 256
NCORES = 8
PB = B // NCORES          # 128 trajectories per core
G = 2                     # pipelined groups per core
GB = PB // G              # 64 trajectories per group
CH = 64                   # steps per z/xdelta chunk
NSTEPS = T - 1
SCALE = 64.0              # xdelta fp16 scaling

LAST_RESULTS = None


def _build(nsteps):
    nc = bacc.Bacc("TRN2", target_bir_lowering=False, debug=False,
                   num_devices=NCORES)
    nchunks = (nsteps + CH - 1) // CH
    nslot = nchunks * CH

    d = {}
    def din(name, shape, dt):
        d[name] = nc.dram_tensor(name, shape, dt, kind="ExternalInput").ap()

    for g in range(G):
        din(f"zpair{g}", [17, nslot * GB], F16)
        din(f"initr{g}", [26, GB], F16)
        din(f"z0r{g}", [8, GB], F16)
    din("updh0", [128, 128], F16)
    din("updh1", [128, 128], F16)
    din("inith0", [26, 128], F16)
    din("inith1", [26, 128], F16)
    din("uzh0", [8, 128], F16)
    din("uzh1", [8, 128], F16)
    din("w2p", [128, 512], F16)
    din("w3p", [128, 512], F16)
    din("w41p", [128, 512], F16)   # (mean_dt * W4 @ Ux) packed like w2p
    din("b2cat", [128, 128], F16)
    din("b3cat", [128, 128], F16)
    din("sel4", [128, 128], F16)
    hout_d = nc.dram_tensor("hout_d", [128, nslot, G, 128], F16,
                            kind="ExternalOutput").ap()

    with TileContext(nc) as tc:
        with tc.tile_pool(name="const", bufs=1) as cpool, \
             tc.tile_pool(name="work", bufs=2) as wpool, \
             tc.tile_pool(name="psum", bufs=1, space="PSUM") as ppool:

            sb = {}
            for name in d:
                if name.startswith("zpair"):
                    continue        # stays in DRAM; chunk-DMA'd below
                shape = [int(s) for s in d[name].shape]
                sb[name] = cpool.tile(shape, d[name].dtype, name=name,
                                      tag=name)
                nc.sync.dma_start(out=sb[name][:], in_=d[name])

            # persistent PSUM banks + stream tiles per group; a2 rotates
            # over 2 banks (8 total: 2 a1 + 4 a2 + 2 a3) so next step's
            # layer-2 bias matmul never waits on this step's elu2 reads
            a1, a2, a3 = {}, {}, {}
            streams = {}
            for g in range(G):
                a1[g] = ppool.tile([128, 128], F32, name=f"a1g{g}",
                                   tag=f"a1g{g}")
                a3[g] = ppool.tile([128, 128], F32, name=f"a3g{g}",
                                   tag=f"a3g{g}")
                for l in (1, 2, 3):
                    for s in ("r", "e", "m"):
                        if l == 3 and s == "m":
                            continue
                        streams[(g, l, s)] = wpool.tile(
                            [128, 128], F16, name=f"{s}{l}g{g}",
                            tag=f"{s}{l}g{g}", bufs=1)

            def emit_step(g, i, blk_tile, blk, cur_tile, i_loc, part):
                a1g = a1[g]
                if part == 1:
                    a2[g] = ppool.tile([128, 128], F32, name=f"a2g{g}",
                                       tag=f"a2g{g}", bufs=2)
                if part == 1 and i == 0:
                    nc.tensor.matmul(a1g[:, 0:GB], lhsT=sb["inith0"][:],
                                     rhs=sb[f"initr{g}"][:],
                                     start=True, stop=False,
                                     skip_group_check=True)
                    nc.tensor.matmul(a1g[:, GB:2 * GB], lhsT=sb["inith1"][:],
                                     rhs=sb[f"initr{g}"][:],
                                     start=False, stop=False,
                                     skip_group_check=True)
                    nc.tensor.matmul(a1g[:, 0:GB], lhsT=sb["uzh0"][:],
                                     rhs=sb[f"z0r{g}"][:],
                                     start=False, stop=False,
                                     skip_group_check=True)
                    nc.tensor.matmul(a1g[:, GB:2 * GB], lhsT=sb["uzh1"][:],
                                     rhs=sb[f"z0r{g}"][:],
                                     start=False, stop=True,
                                     skip_group_check=True)
                elif part == 1 and i > 0:
                    rb = blk_tile[0:128, blk * GB:(blk + 1) * GB]
                    nc.tensor.matmul(a1g[:, 0:GB], lhsT=sb["updh0"][:],
                                     rhs=rb, start=False, stop=False,
                                     skip_group_check=True)
                    nc.tensor.matmul(a1g[:, GB:2 * GB], lhsT=sb["updh1"][:],
                                     rhs=rb, start=False, stop=True,
                                     skip_group_check=True)

                # layer 1..3: streams + next-layer matmuls
                # h' = elu(a)+1 = relu(a) + min(exp(a),1); computed as
                # r (vector), e (scalar), then h' = (e min 1) + r (vector stt)
                layers = ((a1g, a2[g], "w2p", "b2cat"),
                          (a2[g], a3[g], "w3p", "b3cat"),
                          (a3[g], None, "w4p", None))
                sel = (1,) if part == 1 else (2, 3)
                for l, (ain, aout, wname, bname) in enumerate(layers, start=1):
                    if l not in sel:
                        continue
                    r = streams[(g, l, "r")]
                    e = streams[(g, l, "e")]
                    if l == 3:
                        hh = wpool.tile([128, 128], F16, name=f"m3g{g}",
                                        tag=f"m3g{g}", bufs=2)
                    else:
                        hh = streams[(g, l, "m")]
                    nc.vector.tensor_scalar_max(r[:], ain[:], 0.0)
                    nc.scalar.activation(e[:], ain[:], AF.Exp)
                    nc.vector.scalar_tensor_tensor(
                        out=hh[:], in0=e[:], scalar=1.0, in1=r[:],
                        op0=mybir.AluOpType.min, op1=mybir.AluOpType.add)
                    if aout is not None:
                        nc.tensor.matmul(aout[:, 0:128], lhsT=sb[bname][:],
                                         rhs=sb["sel4"][:],
                                         start=True, stop=False)
                        for h in range(2):
                            hs = slice(h * GB, (h + 1) * GB)
                            for kc in range(2):
                                lh = sb[wname][:, (kc * 2 + h) * 128:
                                               (kc * 2 + h + 1) * 128]
                                ks = slice(kc * GB, (kc + 1) * GB)
                                nc.tensor.matmul(aout[:, hs], lhsT=lh,
                                                 rhs=hh[:, ks],
                                                 start=False,
                                                 stop=(h == 1 and kc == 1))
                    else:
                        # critical path: feed x-increment straight into the
                        # a1 recurrence via W41 = mean_dt*W4@Ux; layer 4
                        # itself runs on the host from the shipped h3'
                        for h in range(2):
                            hs = slice(h * GB, (h + 1) * GB)
                            for kc in range(2):
                                lh = sb["w41p"][:, (kc * 2 + h) * 128:
                                                (kc * 2 + h + 1) * 128]
                                ks = slice(kc * GB, (kc + 1) * GB)
                                nc.tensor.matmul(
                                    a1g[:, hs], lhsT=lh, rhs=hh[:, ks],
                                    start=False, stop=False,
                                    skip_group_check=True)
                        nc.sync.dma_start(out=hout_d[:, i, g, :],
                                          in_=hh[:])

            zxt = {g: None for g in range(G)}
            prev = {g: None for g in range(G)}
            for c in range(nchunks):
                for g in range(G):
                    prev[g] = zxt[g]
                    zxt[g] = wpool.tile([128, CH * GB], F16, name=f"zx{g}",
                                        tag=f"zx{g}", bufs=2)
                    if c < 2:
                        # rows 25-127 must be zero for the K=128 update
                        # matmul; clear the whole slot once (real rows are
                        # overwritten by the DMA below / scalar per step)
                        nc.vector.memset(zxt[g][:], 0)
                    nc.sync.dma_start(
                        out=zxt[g][8:25, :],
                        in_=d[f"zpair{g}"][:, c * CH * GB:(c + 1) * CH * GB])
                for i_loc in range(CH):
                    i = c * CH + i_loc
                    if i >= nsteps:
                        break
                    if i_loc == 0:
                        bt0, blk0 = prev[0], CH - 1
                        bt1, blk1 = prev[1], CH - 1
                    else:
                        bt0, blk0 = zxt[0], i_loc - 1
                        bt1, blk1 = zxt[1], i_loc - 1
                    # stagger group 1 half a step behind group 0 so their
                    # serial chains anti-phase instead of colliding on the
                    # same engine at every layer
                    emit_step(0, i, bt0, blk0, zxt[0], i_loc, 1)
                    if i > 0:
                        pl = (i - 1) % CH
                        emit_step(1, i - 1, None, None, pzx1, pl, 2)
                    emit_step(0, i, bt0, blk0, zxt[0], i_loc, 2)
                    emit_step(1, i, bt1, blk1, zxt[1], i_loc, 1)
                    pzx1 = zxt[1]
            # tail: close group 1's final step
            last = nsteps - 1
            emit_step(1, last, None, None, pzx1, last % CH, 2)

    nc.compile()
    return nc


_BUILD_CACHE = {}


def _get_compiled(nsteps):
    if nsteps not in _BUILD_CACHE:
        _BUILD_CACHE[nsteps] = _build(nsteps)
    return _BUILD_CACHE[nsteps]


def _hilo(v):
    hi = v.astype(np.float16)
    lo = (v - hi.astype(np.float32)).astype(np.float16)
    return hi, lo


def kernel(t, x, z, event_t, z_jump, W1, b1, W2, b2, W3, b3, W4, b4,
           nsteps=NSTEPS):
    global LAST_RESULTS
    t = np.asarray(t, np.float32); x = np.asarray(x, np.float32)
    z = np.asarray(z, np.float32)
    event_t = np.asarray(event_t, np.float32)
    z_jump = np.asarray(z_jump, np.float32)
    W1 = np.asarray(W1, np.float32); b1 = np.asarray(b1, np.float32)
    W2 = np.asarray(W2, np.float32); b2 = np.asarray(b2, np.float32)
    W3 = np.asarray(W3, np.float32); b3 = np.asarray(b3, np.float32)
    W4 = np.asarray(W4, np.float32); b4 = np.asarray(b4, np.float32)

    nchunks = (nsteps + CH - 1) // CH
    nslot = nchunks * CH
    tv = t[0, :, 0]
    dt = (tv[1:nsteps + 1] - tv[:nsteps]).astype(np.float32)   # [nsteps]

    # weight-derived shared tensors
    W1a, W1b, W1c = W1[0:16], W1[16:32], W1[32:48]
    V = (W1a - W1b).astype(np.float32)          # [16, 256]
    U = (W1b + W1c).astype(np.float32)          # [16, 256]
    Ux, Uz = U[0:8], U[8:16]
    b1hi, b1lo = _hilo(b1)

    sel4 = np.zeros((128, 128), np.float16)
    sel4[0:2, 0:64] = 1.0
    sel4[2:4, 64:128] = 1.0
    shared = dict(w2p=W2.reshape(2, 128, 2, 128).transpose(1, 0, 2, 3)
                        .reshape(128, 512).astype(np.float16),
                  w3p=W3.reshape(2, 128, 2, 128).transpose(1, 0, 2, 3)
                        .reshape(128, 512).astype(np.float16),
                  sel4=sel4)
    b4eff = (b4 - W4.sum(0)).astype(np.float32)          # [8]
    b4ux = b4eff @ Ux                                    # [256]
    dtbar = np.float32(dt.astype(np.float64).mean())
    W41 = (dtbar * (W4 @ Ux)).astype(np.float32)         # [256, 256]
    shared["w41p"] = (W41.reshape(2, 128, 2, 128).transpose(1, 0, 2, 3)
                      .reshape(128, 512).astype(np.float16))
    for h in range(2):
        hs = slice(h * 128, (h + 1) * 128)
        upd = np.zeros((128, 128), np.float32)
        # rows 0-7 stay zero: the x-increment enters a1 via the fused
        # W41 matmuls; the zx xdelta rows are output-only
        upd[8:16] = Uz[:, hs]
        upd[16:24] = -Uz[:, hs]
        upd[24] = b4ux[hs] / SCALE
        shared[f"updh{h}"] = upd.astype(np.float16)
        init = np.concatenate([Ux[:, hs], V[:, hs],
                               b1hi[None, hs], b1lo[None, hs]], axis=0)
        shared[f"inith{h}"] = init.astype(np.float16)
        shared[f"uzh{h}"] = Uz[:, hs].astype(np.float16)
    for (Wm, bm, name) in ((W2, b2, "b2cat"), (W3, b3, "b3cat")):
        beff = bm - Wm.sum(0)
        hi, lo = _hilo(beff)
        cat = np.zeros((128, 128), np.float16)
        cat[0] = hi[0:128]; cat[1] = lo[0:128]
        cat[2] = hi[128:256]; cat[3] = lo[128:256]
        shared[name] = cat

    in_maps = []
    for c in range(NCORES):
        m = dict(shared)
        for g in range(G):
            bs = slice(c * PB + g * GB, c * PB + (g + 1) * GB)
            mask = tv[None, :nsteps] >= event_t[bs]            # [GB, nsteps]
            zeff = np.where(mask[..., None], z_jump[bs][:, None, :],
                            z[bs, :nsteps]).astype(np.float32)  # [GB,ns,8]
            zp = np.zeros((17, nslot, GB), np.float32)
            zp[8:16, :nsteps] = zeff.transpose(2, 1, 0)         # zeff_i
            zp[0:8, :nsteps - 1] = zeff[:, 1:].transpose(2, 1, 0)
            zp[16, :nsteps] = (SCALE * dt)[:, None]             # b4 row
            m[f"zpair{g}"] = zp.reshape(17, nslot * GB).astype(np.float16)
            x0 = x[bs, 0]                                       # [GB, 8]
            z0 = z[bs, 0]
            initr = np.concatenate(
                [x0.T, x0.T, z0.T, np.ones((2, GB), np.float32)], axis=0)
            m[f"initr{g}"] = initr.astype(np.float16)
            m[f"z0r{g}"] = zeff[:, 0].T.astype(np.float16)
        in_maps.append({k: np.ascontiguousarray(v) for k, v in m.items()})

    nc = _get_compiled(nsteps)
    res = bass_utils.run_bass_kernel_spmd(nc, in_maps,
                                          core_ids=list(range(NCORES)))
    LAST_RESULTS = res

    out = np.zeros((B, T, XD), np.float32)
    n = min(nsteps + 1, T)
    for c in range(NCORES):
        raw = res.results[c]["hout_d"]          # [128, nslot, G, 128] f16
        for g in range(G):
            bs = slice(c * PB + g * GB, c * PB + (g + 1) * GB)
            # cols = (half, batch): H[half*128+p, i, b] = raw[p, i, g, h*64+b]
            hg = raw[:, :nsteps, g, :].astype(np.float32)   # [128, ns, 128]
            H = np.concatenate([hg[:, :, 0:GB], hg[:, :, GB:2 * GB]],
                               axis=0)                      # [256, ns, GB]
            dx = np.einsum('kd,kib->dib', W4, H) + b4eff[:, None, None]
            xd = dx * dt[None, :, None]                     # [8, ns, GB]
            cum = np.cumsum(xd, axis=1)
            out[bs, 0] = x[bs, 0]
            out[bs, 1:n] = (x[bs, 0][:, None, :]
                            + cum.transpose(2, 1, 0)[:, :n - 1])
    return out
